# revision 21
# baseline (speedup 1.0000x reference)
"""KGAT recommender (3-layer GNN message passing) on 8 Trainium2 NeuronCores.

Sharding: edges are sharded by destination-node range — core k owns nodes
[k*12500, (k+1)*12500) and aggregates all messages into them; each layer ends
with an AllGather of the updated (bf16) node-embedding table (x rows carry the
per-node attention scalar s=x@Wa_top+ba at col 128, so edge gathers fetch it
for free).

Per-edge source rows are fetched with dma_gather (one Pool instruction per
(region, src-quarter) instead of one indirect DMA per 128-edge chunk).
dma_gather indices are int16, so the 100352-row replicated table is addressed
as 4 quarters of 25088 rows; edges are grouped (window, quarter) with a
uniform CU chunks per (window, quarter) cell so each gather call covers a
contiguous chunk range and all per-window AP arithmetic stays affine.

Attention: att = sigmoid(s[src] + t[dst]).  t is never gathered per edge:
per dst window a rank-1 matmul (ones^T @ diag(t_w)) broadcasts t_w across
partitions, and per (region, quarter) group ONE DVE pass builds the one-hot
OH[e,c,j] = (j == dla[e,c]), one pass PRE = T_bc + s, one ACT pass
SIG = sigmoid(PRE), one pass W = OH*SIG — then PSUM accumulates
aggT[:, w] += G[:, c, 0:128]^T @ W[:, c, :] over the window's 8 chunks.
"""

import os
import numpy as np
import ml_dtypes

import concourse.bacc as bacc
import concourse.bass as bass
import concourse.mybir as mybir
import concourse.tile as tile
from concourse.bass_utils import run_bass_kernel_spmd
from concourse.masks import make_identity

BF16 = ml_dtypes.bfloat16

NCORES = 8
N = 100000
U = 50000
D = 128
L = 3
P = 128
NPC = N // NCORES          # 12500 nodes per core
WPC = (NPC + P - 1) // P   # 98 windows per core
NSLAB = WPC * P            # 12544 padded rows per core
TAB = NCORES * NSLAB       # 100352 rows in the gather table
NQ = 4                     # src quarters (int16 index limit)
QROWS = TAB // NQ          # 25088 rows per quarter
ROWC = 256                 # table row: [x bf16 x128 | s bf16 | pad]

LAST_EXEC_NS = None


def _host_prep(edge_index, user_emb, item_emb, Wa, ba, Wg, bg):
    x0 = np.concatenate([np.asarray(user_emb), np.asarray(item_emb)], 0).astype(np.float32)
    Wa = np.asarray(Wa, np.float32)
    ba = np.asarray(ba, np.float32)
    Wg = np.asarray(Wg, np.float32)
    bg = np.asarray(bg, np.float32)

    src = np.asarray(edge_index[0]).astype(np.int64)
    dst = np.asarray(edge_index[1]).astype(np.int64)

    core = dst // NPC
    local = dst % NPC
    w_of = local // P
    dloc = local % P
    tabrow = (src // NPC) * NSLAB + (src % NPC)
    quarter = tabrow // QROWS
    qrow = tabrow % QROWS

    cnt = np.zeros((NCORES, WPC, NQ), np.int64)
    np.add.at(cnt, (core, w_of, quarter), 1)
    CU = int(np.ceil(cnt.max() / P))           # uniform chunks per (w, q) cell

    RW = int(os.environ.get("KGAT_RW", "7"))
    # region boundaries and per-region chunk bases
    rstarts = list(range(0, WPC, RW))
    rof = np.zeros(WPC, np.int64)     # region index of window
    wbase = np.zeros(WPC, np.int64)   # first window of the region
    chbase = np.zeros(len(rstarts), np.int64)
    ch = 0
    regions = []
    for r, w0 in enumerate(rstarts):
        w1 = min(w0 + RW, WPC)
        chbase[r] = ch
        regions.append(dict(w0=w0, w1=w1, ch0=ch, rwr=w1 - w0))
        rof[w0:w1] = r
        wbase[w0:w1] = w0
        ch += (w1 - w0) * NQ * CU
    TOTCH = ch

    def chunk_id(w, q, j):
        r = rof[w]
        rwr = regions[r]["w1"] - regions[r]["w0"]
        return chbase[r] + q * rwr * CU + (w - wbase[w]) * CU + j

    idx16 = np.zeros((NCORES, 16, TOTCH * 8), np.int16)
    dla = np.full((NCORES, P, TOTCH), 300.0, np.float32)
    key = (core * WPC + w_of) * NQ + quarter
    order = np.argsort(key, kind="stable")
    ks, ws, qs = core[order], w_of[order], quarter[order]
    key_s = key[order]
    starts = np.searchsorted(key_s, np.arange(NCORES * WPC * NQ))
    rank = np.arange(len(order)) - starts[key_s]
    rwr_w = np.asarray([regions[rof[w]]["w1"] - regions[rof[w]]["w0"]
                        for w in range(WPC)], np.int64)
    # u-major chunk order within each (region, quarter) group keeps every
    # DVE grid AP packed in its last dim (stride 1 over w)
    chunk_g = (chbase[rof[ws]] + qs * rwr_w[ws] * CU + (rank // P) * rwr_w[ws]
               + (ws - wbase[ws]))
    p_slot = rank % P
    k_flat = chunk_g * P + p_slot
    idx16[ks, k_flat % 16, k_flat // 16] = qrow[order].astype(np.int16)
    dla[ks, p_slot, chunk_g] = dloc[order].astype(np.float32)
    idx16 = np.tile(idx16, (1, 8, 1))

    s0 = x0 @ Wa[0, :D, 0] + ba[0, 0]
    t0 = x0 @ Wa[0, D:, 0]

    xslab = np.zeros((NCORES, NSLAB, ROWC), BF16)
    t0c = np.zeros((NCORES, P, WPC), np.float32)
    xt0 = np.zeros((NCORES, P, NSLAB), BF16)
    for k in range(NCORES):
        xslab[k, :NPC, :D] = x0[k * NPC:(k + 1) * NPC].astype(BF16)
        xslab[k, :NPC, D] = s0[k * NPC:(k + 1) * NPC].astype(BF16)
        tk = np.zeros(NSLAB, np.float32)
        tk[:NPC] = t0[k * NPC:(k + 1) * NPC]
        t0c[k] = tk.reshape(WPC, P).T
        xp = np.zeros((NSLAB, D), np.float32)
        xp[:NPC] = x0[k * NPC:(k + 1) * NPC]
        xt0[k] = np.ascontiguousarray(xp.T).astype(BF16)

    wg_b = np.zeros((L, 2, D, D), BF16)
    for l in range(L):
        wg_b[l, 0] = Wg[l, :D].astype(BF16)
        wg_b[l, 1] = Wg[l, D:].astype(BF16)
    wast = np.zeros((L - 1, D, 2), BF16)
    for l in range(1, L):
        wast[l - 1, :, 0] = Wa[l, :D, 0].astype(BF16)
        wast[l - 1, :, 1] = Wa[l, D:, 0].astype(BF16)
    bg_c = bg.reshape(L, D, 1).astype(np.float32)

    layout = dict(TOTCH=TOTCH, CU=CU, regions=regions)
    return dict(layout=layout, idx16=idx16, dla=dla, xslab=xslab, t0c=t0c,
                xt0=xt0, wg_b=wg_b, wast=wast, bg_c=bg_c, ba=ba)


def _build_nc(layout, ba):
    L_RUN = int(os.environ.get("KGAT_LAYERS", str(L)))
    REPS = int(os.environ.get("KGAT_REPS", "1"))
    TOTCH = layout["TOTCH"]
    CU = layout["CU"]
    regions = layout["regions"]
    dt = mybir.dt
    nc = bacc.Bacc("TRN2", target_bir_lowering=False, debug=False,
                   enable_asserts=False, num_devices=NCORES)

    i_xslab = nc.dram_tensor("xslab", [NSLAB, ROWC], dt.bfloat16, kind="ExternalInput")
    i_xt0 = nc.dram_tensor("xt0", [P, NSLAB], dt.bfloat16, kind="ExternalInput")
    i_t0 = nc.dram_tensor("t0", [P, WPC], dt.float32, kind="ExternalInput")
    i_idx16 = nc.dram_tensor("idx16", [P, TOTCH * 8], dt.int16, kind="ExternalInput")
    i_dla = nc.dram_tensor("dla", [P, TOTCH], dt.float32, kind="ExternalInput")
    i_wg = nc.dram_tensor("wg", [L, 2, D, D], dt.bfloat16, kind="ExternalInput")
    i_wast = nc.dram_tensor("wast", [L - 1, D, 2], dt.bfloat16, kind="ExternalInput")
    i_bg = nc.dram_tensor("bg", [L, D, 1], dt.float32, kind="ExternalInput")
    o_out = nc.dram_tensor("out", [NSLAB, D], dt.float32, kind="ExternalOutput")

    agin = [nc.dram_tensor(f"agin{l}", [NSLAB, ROWC], dt.bfloat16, kind="Internal")
            for l in range(L)]
    xfull = [nc.dram_tensor(f"xfull{l}", [TAB, ROWC], dt.bfloat16, kind="Internal",
                            addr_space="Shared")
             for l in range(L)]

    with tile.TileContext(nc) as tc:
        with (
            tc.tile_pool(name="sb", bufs=1) as sb,
            tc.tile_pool(name="sbg", bufs=2) as sbg,
            tc.tile_pool(name="sbq", bufs=2) as sbq,
            tc.tile_pool(name="sbw", bufs=3) as sbw,
            tc.tile_pool(name="ps", bufs=2, space="PSUM") as ps,
            tc.tile_pool(name="ps1", bufs=1, space="PSUM") as ps1,
            tc.tile_pool(name="psT", bufs=1, space="PSUM") as psT,
        ):
            # ---- constants / persistent state ----
            iota_i = sb.tile([P, P], dt.int32)
            nc.gpsimd.iota(iota_i[:], pattern=[[1, P]], base=0, channel_multiplier=0)
            iota_b = sb.tile([P, P], dt.bfloat16)
            nc.vector.tensor_copy(out=iota_b[:], in_=iota_i[:])
            iotac_i = sb.tile([P, 1], dt.int32)
            nc.gpsimd.iota(iotac_i[:], pattern=[[0, 1]], base=0, channel_multiplier=1)
            iotac_f = sb.tile([P, 1], dt.float32)
            nc.vector.tensor_copy(out=iotac_f[:], in_=iotac_i[:])
            ones_b = sb.tile([P, P], dt.bfloat16)
            nc.vector.memset(ones_b[:], 1.0)
            ident_b = sb.tile([P, P], dt.bfloat16)
            make_identity(nc, ident_b[:])
            ident_f = sb.tile([P, P], dt.float32)
            make_identity(nc, ident_f[:])

            idx16_sb = sb.tile([P, TOTCH * 8], dt.int16)
            nc.sync.dma_start(out=idx16_sb[:], in_=i_idx16.ap())
            dla_sb = sb.tile([P, TOTCH], dt.float32)
            nc.sync.dma_start(out=dla_sb[:], in_=i_dla.ap())
            dlab_sb = sb.tile([P, TOTCH], dt.bfloat16)
            nc.vector.tensor_copy(out=dlab_sb[:], in_=dla_sb[:])

            RWR = regions[0]["rwr"]
            CHGC = RWR * CU
            # static [p, j, c] grid: value j, packed along c
            iotajc = sb.tile([P, P, CHGC], dt.bfloat16)
            nc.vector.tensor_copy(
                out=iotajc[:],
                in_=iota_b[:].rearrange("p (j o) -> p j o", o=1)
                    .to_broadcast([P, P, CHGC]))

            wg_sb = sb.tile([P, L * 2 * D], dt.bfloat16)
            for l in range(L):
                for h in range(2):
                    nc.sync.dma_start(out=wg_sb[:, (l * 2 + h) * D:(l * 2 + h + 1) * D],
                                      in_=i_wg.ap()[l, h])
            wast_sb = sb.tile([P, (L - 1) * 2], dt.bfloat16)
            for l in range(L - 1):
                nc.sync.dma_start(out=wast_sb[:, l * 2:l * 2 + 2], in_=i_wast.ap()[l])
            bg_sb = sb.tile([P, L], dt.float32)
            for l in range(L):
                nc.sync.dma_start(out=bg_sb[:, l:l + 1], in_=i_bg.ap()[l])

            xt_own = sb.tile([P, NSLAB], dt.bfloat16)
            tstages = [sb.tile([P, WPC], dt.float32, tag=f"tst{l}", name=f"tst{l}")
                       for l in range(L)]


            NO_COLL = int(os.environ.get("KGAT_NO_COLL", "0"))

            def allgather(src_t, dst_t):
                if NO_COLL == 2:
                    # light stub for TimelineSim: model only the local write;
                    # the real collective runs on TOPSP/SDMA, not our engines
                    nc.sync.dma_start(out=dst_t.ap()[0:NSLAB], in_=src_t.ap())
                elif NO_COLL:
                    for k in range(NCORES):
                        nc.sync.dma_start(
                            out=dst_t.ap()[k * NSLAB:(k + 1) * NSLAB], in_=src_t.ap())
                else:
                    nc.gpsimd.collective_compute(
                        "AllGather", mybir.AluOpType.bypass,
                        replica_groups=[list(range(NCORES))],
                        ins=[src_t.ap()], outs=[dst_t.ap()])

            for rep in range(REPS):
                nc.sync.dma_start(out=xt_own[:], in_=i_xt0.ap())
                nc.sync.dma_start(out=tstages[0][:], in_=i_t0.ap())
                nc.sync.dma_start(out=agin[0].ap(), in_=i_xslab.ap())
                allgather(agin[0], xfull[0])

                for l in range(L_RUN):
                    last = (l == L_RUN - 1)
                    xsrc = xfull[l]
                    tst = tstages[l]
                    if not last:
                        stage = sb.tile([P, WPC, ROWC], dt.bfloat16, tag="stage")
                        nc.vector.memset(stage[:, :, D + 1:], 0)
                    else:
                        stagef = sb.tile([P, WPC, D], dt.float32, tag="stage")

                    for reg in regions:
                        w0, w1, ch0, rwr = reg["w0"], reg["w1"], reg["ch0"], reg["rwr"]
                        CHG = rwr * CU
                        CHR = CHG * NQ
                        G = sbg.tile([P, CHR, ROWC], dt.bfloat16, tag="G")
                        for q in range(NQ):
                            cha = ch0 + q * CHG
                            nc.gpsimd.dma_gather(
                                G[:, q * CHG:(q + 1) * CHG, :],
                                xsrc.ap()[q * QROWS:(q + 1) * QROWS],
                                idx16_sb[:, cha * 8:(cha + CHG) * 8],
                                CHG * P, CHG * P, ROWC, single_packet=False)

                        # TBCJ[p, j, w] = t_w[j]: per-window rank-1 matmul then copy
                        TBCJ = sbg.tile([P, P, rwr], dt.bfloat16, tag="TBCJ")
                        for w in range(w0, w1):
                            diag = sbw.tile([P, P], dt.bfloat16, tag="diag")
                            nc.vector.tensor_scalar(
                                diag[:], iota_b[:], iotac_f[:, 0:1], tst[:, w:w + 1],
                                mybir.AluOpType.is_equal, mybir.AluOpType.mult)
                            tb = psT.tile([P, P], dt.float32, tag="tb")
                            nc.tensor.matmul(out=tb[:], lhsT=ones_b[:], rhs=diag[:],
                                             start=True, stop=True)
                            nc.vector.tensor_copy(
                                out=TBCJ[:, :, w - w0:w - w0 + 1],
                                in_=tb[:].rearrange("p (j o) -> p j o", o=1))

                        aggP = ps.tile([P, rwr * P], dt.float32, tag="agg")
                        Wts = []
                        for q in range(NQ):
                            base = q * CHG
                            gsl = G[:, base:base + CHG, :]
                            # s per chunk, packed [P, CHG]
                            scol = sbw.tile([P, CHG], dt.bfloat16, tag="scol")
                            nc.vector.tensor_copy(
                                out=scol[:],
                                in_=gsl[:, :, D:D + 1].rearrange("p c o -> p (c o)"))
                            OH = sbq.tile([P, P, CHG], dt.bfloat16, tag="OH")
                            nc.vector.tensor_tensor(
                                out=OH[:],
                                in0=iotajc[:, :, :CHG],
                                in1=dlab_sb[:, ch0 + base:ch0 + base + CHG]
                                    .rearrange("p (o c) -> p o c", o=1)
                                    .to_broadcast([P, P, CHG]),
                                op=mybir.AluOpType.is_equal)
                            PRE = sbq.tile([P, P, CHG], dt.bfloat16, tag="PRE")
                            nc.vector.tensor_tensor(
                                out=PRE[:].rearrange("p j (u w) -> p j u w", u=CU),
                                in0=TBCJ[:].rearrange("p j (o w) -> p j o w", o=1)
                                    .to_broadcast([P, P, CU, rwr]),
                                in1=scol[:].rearrange("p (o u w) -> p o u w", o=1, u=CU)
                                    .to_broadcast([P, P, CU, rwr]),
                                op=mybir.AluOpType.add)
                            SIG = sbq.tile([P, P, CHG], dt.bfloat16, tag="SIG")
                            nc.scalar.activation(
                                out=SIG[:], in_=PRE[:],
                                func=mybir.ActivationFunctionType.Sigmoid)
                            Wt = sbq.tile([P, P, CHG], dt.bfloat16, tag="W",
                                          bufs=NQ + 1)
                            nc.vector.tensor_tensor(
                                out=Wt[:], in0=OH[:], in1=SIG[:],
                                op=mybir.AluOpType.mult)
                            Wts.append((base, Wt))
                        for w in range(w0, w1):
                            for q in range(NQ):
                                base, Wt = Wts[q]
                                for u in range(CU):
                                    c = u * rwr + (w - w0)
                                    nc.tensor.matmul(
                                        out=aggP[:, (w - w0) * P:(w - w0 + 1) * P],
                                        lhsT=G[:, base + c, 0:D], rhs=Wt[:, :, c],
                                        start=(q == 0 and u == 0),
                                        stop=(q == NQ - 1 and u == CU - 1))

                        # ---- node updates for the region's windows ----
                        for w in range(w0, w1):
                            aggb = sbw.tile([P, P], dt.bfloat16, tag="aggb")
                            nc.vector.tensor_copy(
                                out=aggb[:], in_=aggP[:, (w - w0) * P:(w - w0 + 1) * P])
                            xts = xt_own[:, w * P:(w + 1) * P]
                            up = ps1.tile([P, P], dt.float32, tag="up")
                            nc.tensor.matmul(out=up[:],
                                             lhsT=wg_sb[:, (l * 2) * D:(l * 2 + 1) * D],
                                             rhs=xts, start=True, stop=False)
                            nc.tensor.matmul(out=up[:],
                                             lhsT=wg_sb[:, (l * 2 + 1) * D:(l * 2 + 2) * D],
                                             rhs=aggb[:], start=False, stop=True)
                            if not last:
                                nc.scalar.activation(out=xts, in_=up[:],
                                                     func=mybir.ActivationFunctionType.Relu,
                                                     bias=bg_sb[:, l:l + 1])
                                st = ps1.tile([P, 2], dt.float32, tag="st")
                                nc.tensor.matmul(out=st[:], lhsT=xts,
                                                 rhs=wast_sb[:, l * 2:l * 2 + 2],
                                                 start=True, stop=True)
                                tr = ps1.tile([P, P], dt.bfloat16, tag="tr")
                                nc.tensor.transpose(out=tr[:], in_=xts,
                                                    identity=ident_b[:])
                                nc.vector.tensor_copy(out=stage[:, w, 0:D], in_=tr[:])
                                nc.scalar.add(out=stage[:, w, D:D + 1], in_=st[:, 0:1],
                                              add=float(ba[l + 1, 0]))
                                nc.vector.tensor_copy(out=tstages[l + 1][:, w:w + 1],
                                                      in_=st[:, 1:2])
                            else:
                                xf = sbw.tile([P, P], dt.float32, tag="xf")
                                nc.scalar.activation(out=xf[:], in_=up[:],
                                                     func=mybir.ActivationFunctionType.Relu,
                                                     bias=bg_sb[:, l:l + 1])
                                trf = ps1.tile([P, P], dt.float32, tag="tr")
                                nc.tensor.transpose(out=trf[:], in_=xf[:],
                                                    identity=ident_f[:])
                                nc.vector.tensor_copy(out=stagef[:, w, :], in_=trf[:])

                    if not last:
                        nc.sync.dma_start(
                            out=agin[l + 1].ap().rearrange("(w p) c -> p w c", p=P),
                            in_=stage[:])
                        allgather(agin[l + 1], xfull[l + 1])
                    else:
                        nc.sync.dma_start(
                            out=o_out.ap().rearrange("(w p) c -> p w c", p=P),
                            in_=stagef[:])

    nc.compile()
    return nc


def kernel(edge_index, user_emb, item_emb, Wa, ba, Wg, bg):
    global LAST_EXEC_NS
    h = _host_prep(edge_index, user_emb, item_emb, Wa, ba, Wg, bg)
    nc = _build_nc(h["layout"], h["ba"])

    in_maps = []
    for k in range(NCORES):
        in_maps.append({
            "xslab": h["xslab"][k], "xt0": h["xt0"][k], "t0": h["t0c"][k],
            "idx16": h["idx16"][k], "dla": h["dla"][k],
            "wg": h["wg_b"], "wast": h["wast"], "bg": h["bg_c"],
        })

    res = run_bass_kernel_spmd(nc, in_maps, core_ids=list(range(NCORES)))
    LAST_EXEC_NS = res.exec_time_ns

    if int(os.environ.get("KGAT_BENCH", "0")):
        LAST_EXEC_NS = _bench(nc, in_maps)

    x = np.zeros((N, D), np.float32)
    for k in range(NCORES):
        x[k * NPC:(k + 1) * NPC] = np.asarray(res.results[k]["out"])[:NPC]
    return x[:U], x[U:]


def _bench(nc, in_maps, iters=None):
    """Time repeated on-device executions via the same PJRT shard_map path
    (device-resident inputs, no donation) and return min wall ns."""
    import time
    import jax
    from jax.sharding import Mesh, PartitionSpec
    from jax.experimental.shard_map import shard_map
    from concourse import bass2jax, mybir as mb

    if iters is None:
        iters = int(os.environ.get("KGAT_BENCH_ITERS", "10"))

    bass2jax.install_neuronx_cc_hook()
    partition_name = (nc.partition_id_tensor.name
                      if nc.partition_id_tensor else None)
    in_names, out_names, out_avals, zero_outs = [], [], [], []
    for alloc in nc.m.functions[0].allocations:
        if not isinstance(alloc, mb.MemoryLocationSet):
            continue
        name = alloc.memorylocations[0].name
        if alloc.kind == "ExternalInput":
            if name != partition_name:
                in_names.append(name)
        elif alloc.kind == "ExternalOutput":
            out_names.append(name)
            shape = tuple(alloc.tensor_shape)
            dtype = mb.dt.np(alloc.dtype)
            out_avals.append(jax.core.ShapedArray(shape, dtype))
            zero_outs.append(np.zeros(shape, dtype))
    n_params = len(in_names)
    all_names = in_names + out_names
    if partition_name is not None:
        all_names = all_names + [partition_name]

    def _body(*args):
        operands = list(args)
        if partition_name is not None:
            operands.append(bass2jax.partition_id_tensor())
        return tuple(bass2jax._bass_exec_p.bind(
            *operands, out_avals=tuple(out_avals), in_names=tuple(all_names),
            out_names=tuple(out_names), lowering_input_output_aliases=(),
            sim_require_finite=False, sim_require_nnan=False, nc=nc))

    devices = jax.devices()[:NCORES]
    mesh = Mesh(np.asarray(devices), ("core",))
    specs = (PartitionSpec("core"),) * (n_params + len(out_names))
    fn = jax.jit(shard_map(_body, mesh=mesh, in_specs=specs,
                           out_specs=(PartitionSpec("core"),) * len(out_names),
                           check_rep=False), keep_unused=True)
    concat_in = [np.concatenate([np.asarray(m[n]) for m in in_maps], axis=0)
                 for n in in_names]
    concat_zero = [np.zeros((NCORES * z.shape[0], *z.shape[1:]), z.dtype)
                   for z in zero_outs]
    sharding = jax.sharding.NamedSharding(mesh, PartitionSpec("core"))
    dev_in = [jax.device_put(a, sharding) for a in concat_in + concat_zero]
    jax.block_until_ready(fn(*dev_in))  # warm compile
    times = []
    for _ in range(iters):
        t0 = time.perf_counter()
        jax.block_until_ready(fn(*dev_in))
        times.append(time.perf_counter() - t0)
    times.sort()
    print(f"bench iters (ms): {[f'{t*1e3:.2f}' for t in times]}")
    return int(times[0] * 1e9)


# revision 23
# speedup vs baseline: 2.2123x; 2.2123x over previous
"""KGAT recommender (3-layer GNN message passing) on 8 Trainium2 NeuronCores.

Sharding: edges are sharded by destination-node range — core k owns nodes
[k*12500, (k+1)*12500) and aggregates all messages into them; each layer ends
with an AllGather of the updated (bf16) node-embedding table (x rows carry the
per-node attention scalar s=x@Wa_top+ba at col 128, so edge gathers fetch it
for free).

Per-edge source rows are fetched with dma_gather (one Pool instruction per
(region, src-quarter) instead of one indirect DMA per 128-edge chunk).
dma_gather indices are int16, so the 100352-row replicated table is addressed
as 4 quarters of 25088 rows; edges are grouped (window, quarter) with a
uniform CU chunks per (window, quarter) cell so each gather call covers a
contiguous chunk range and all per-window AP arithmetic stays affine.

Attention: att = sigmoid(s[src] + t[dst]).  t is never gathered per edge:
per dst window a rank-1 matmul (ones^T @ diag(t_w)) broadcasts t_w across
partitions, and per (region, quarter) group ONE DVE pass builds the one-hot
OH[e,c,j] = (j == dla[e,c]), one pass PRE = T_bc + s, one ACT pass
SIG = sigmoid(PRE), one pass W = OH*SIG — then PSUM accumulates
aggT[:, w] += G[:, c, 0:128]^T @ W[:, c, :] over the window's 8 chunks.
"""

import os
import numpy as np
import ml_dtypes

import concourse.bacc as bacc
import concourse.bass as bass
import concourse.mybir as mybir
import concourse.tile as tile
from concourse.bass_utils import run_bass_kernel_spmd
from concourse.masks import make_identity

BF16 = ml_dtypes.bfloat16

NCORES = 8
N = 100000
U = 50000
D = 128
L = 3
P = 128
NPC = N // NCORES          # 12500 nodes per core
WPC = (NPC + P - 1) // P   # 98 windows per core
NSLAB = WPC * P            # 12544 padded rows per core
TAB = NCORES * NSLAB       # 100352 rows in the gather table
NQ = 4                     # src quarters (int16 index limit)
QROWS = TAB // NQ          # 25088 rows per quarter
ROWC = 256                 # table row: [x bf16 x128 | s bf16 | pad]

LAST_EXEC_NS = None


def _host_prep(edge_index, user_emb, item_emb, Wa, ba, Wg, bg):
    x0 = np.concatenate([np.asarray(user_emb), np.asarray(item_emb)], 0).astype(np.float32)
    Wa = np.asarray(Wa, np.float32)
    ba = np.asarray(ba, np.float32)
    Wg = np.asarray(Wg, np.float32)
    bg = np.asarray(bg, np.float32)

    src = np.asarray(edge_index[0]).astype(np.int64)
    dst = np.asarray(edge_index[1]).astype(np.int64)

    core = dst // NPC
    local = dst % NPC
    w_of = local // P
    dloc = local % P
    tabrow = (src // NPC) * NSLAB + (src % NPC)
    quarter = tabrow // QROWS
    qrow = tabrow % QROWS

    cnt = np.zeros((NCORES, WPC, NQ), np.int64)
    np.add.at(cnt, (core, w_of, quarter), 1)
    CU = int(np.ceil(cnt.max() / P))           # uniform chunks per (w, q) cell

    RW = int(os.environ.get("KGAT_RW", "7"))
    # region boundaries and per-region chunk bases
    rstarts = list(range(0, WPC, RW))
    rof = np.zeros(WPC, np.int64)     # region index of window
    wbase = np.zeros(WPC, np.int64)   # first window of the region
    chbase = np.zeros(len(rstarts), np.int64)
    ch = 0
    regions = []
    for r, w0 in enumerate(rstarts):
        w1 = min(w0 + RW, WPC)
        chbase[r] = ch
        regions.append(dict(w0=w0, w1=w1, ch0=ch, rwr=w1 - w0))
        rof[w0:w1] = r
        wbase[w0:w1] = w0
        ch += (w1 - w0) * NQ * CU
    TOTCH = ch

    def chunk_id(w, q, j):
        r = rof[w]
        rwr = regions[r]["w1"] - regions[r]["w0"]
        return chbase[r] + q * rwr * CU + (w - wbase[w]) * CU + j

    idx16 = np.zeros((NCORES, 16, TOTCH * 8), np.int16)
    dla = np.full((NCORES, P, TOTCH), 300.0, np.float32)
    key = (core * WPC + w_of) * NQ + quarter
    order = np.argsort(key, kind="stable")
    ks, ws, qs = core[order], w_of[order], quarter[order]
    key_s = key[order]
    starts = np.searchsorted(key_s, np.arange(NCORES * WPC * NQ))
    rank = np.arange(len(order)) - starts[key_s]
    rwr_w = np.asarray([regions[rof[w]]["w1"] - regions[rof[w]]["w0"]
                        for w in range(WPC)], np.int64)
    # u-major chunk order within each (region, quarter) group keeps every
    # DVE grid AP packed in its last dim (stride 1 over w)
    chunk_g = (chbase[rof[ws]] + qs * rwr_w[ws] * CU + (rank // P) * rwr_w[ws]
               + (ws - wbase[ws]))
    p_slot = rank % P
    k_flat = chunk_g * P + p_slot
    idx16[ks, k_flat % 16, k_flat // 16] = qrow[order].astype(np.int16)
    dla[ks, p_slot, chunk_g] = dloc[order].astype(np.float32)
    idx16 = np.tile(idx16, (1, 8, 1))

    s0 = x0 @ Wa[0, :D, 0] + ba[0, 0]
    t0 = x0 @ Wa[0, D:, 0]

    xslab = np.zeros((NCORES, NSLAB, ROWC), BF16)
    t0c = np.zeros((NCORES, P, WPC), np.float32)
    xt0 = np.zeros((NCORES, P, NSLAB), BF16)
    for k in range(NCORES):
        xslab[k, :NPC, :D] = x0[k * NPC:(k + 1) * NPC].astype(BF16)
        xslab[k, :NPC, D] = s0[k * NPC:(k + 1) * NPC].astype(BF16)
        tk = np.zeros(NSLAB, np.float32)
        tk[:NPC] = t0[k * NPC:(k + 1) * NPC]
        t0c[k] = tk.reshape(WPC, P).T
        xp = np.zeros((NSLAB, D), np.float32)
        xp[:NPC] = x0[k * NPC:(k + 1) * NPC]
        xt0[k] = np.ascontiguousarray(xp.T).astype(BF16)

    wg_b = np.zeros((L, 2, D, D), BF16)
    for l in range(L):
        wg_b[l, 0] = Wg[l, :D].astype(BF16)
        wg_b[l, 1] = Wg[l, D:].astype(BF16)
    wast = np.zeros((L - 1, D, 2), BF16)
    for l in range(1, L):
        wast[l - 1, :, 0] = Wa[l, :D, 0].astype(BF16)
        wast[l - 1, :, 1] = Wa[l, D:, 0].astype(BF16)
    bg_c = bg.reshape(L, D, 1).astype(np.float32)

    layout = dict(TOTCH=TOTCH, CU=CU, regions=regions)
    return dict(layout=layout, idx16=idx16, dla=dla, xslab=xslab, t0c=t0c,
                xt0=xt0, wg_b=wg_b, wast=wast, bg_c=bg_c, ba=ba)


def _build_nc(layout, ba):
    L_RUN = int(os.environ.get("KGAT_LAYERS", str(L)))
    REPS = int(os.environ.get("KGAT_REPS", "1"))
    TOTCH = layout["TOTCH"]
    CU = layout["CU"]
    regions = layout["regions"]
    dt = mybir.dt
    nc = bacc.Bacc("TRN2", target_bir_lowering=False, debug=False,
                   enable_asserts=False, num_devices=NCORES)

    i_xslab = nc.dram_tensor("xslab", [NSLAB, ROWC], dt.bfloat16, kind="ExternalInput")
    i_xt0 = nc.dram_tensor("xt0", [P, NSLAB], dt.bfloat16, kind="ExternalInput")
    i_t0 = nc.dram_tensor("t0", [P, WPC], dt.float32, kind="ExternalInput")
    i_idx16 = nc.dram_tensor("idx16", [P, TOTCH * 8], dt.int16, kind="ExternalInput")
    i_dla = nc.dram_tensor("dla", [P, TOTCH], dt.float32, kind="ExternalInput")
    i_wg = nc.dram_tensor("wg", [L, 2, D, D], dt.bfloat16, kind="ExternalInput")
    i_wast = nc.dram_tensor("wast", [L - 1, D, 2], dt.bfloat16, kind="ExternalInput")
    i_bg = nc.dram_tensor("bg", [L, D, 1], dt.float32, kind="ExternalInput")
    o_out = nc.dram_tensor("out", [NSLAB, D], dt.float32, kind="ExternalOutput")

    agin = [nc.dram_tensor(f"agin{l}", [NSLAB, ROWC], dt.bfloat16, kind="Internal")
            for l in range(L)]
    xfull = [nc.dram_tensor(f"xfull{l}", [TAB, ROWC], dt.bfloat16, kind="Internal",
                            addr_space="Shared")
             for l in range(L)]

    with tile.TileContext(nc) as tc:
        with (
            tc.tile_pool(name="sb", bufs=1) as sb,
            tc.tile_pool(name="sbg", bufs=2) as sbg,
            tc.tile_pool(name="sbq", bufs=2) as sbq,
            tc.tile_pool(name="sbw", bufs=3) as sbw,
            tc.tile_pool(name="ps", bufs=2, space="PSUM") as ps,
            tc.tile_pool(name="ps1", bufs=1, space="PSUM") as ps1,
            tc.tile_pool(name="psT", bufs=1, space="PSUM") as psT,
        ):
            # ---- constants / persistent state ----
            iota_i = sb.tile([P, P], dt.int32)
            nc.gpsimd.iota(iota_i[:], pattern=[[1, P]], base=0, channel_multiplier=0)
            iota_b = sb.tile([P, P], dt.bfloat16)
            nc.vector.tensor_copy(out=iota_b[:], in_=iota_i[:])
            iotac_i = sb.tile([P, 1], dt.int32)
            nc.gpsimd.iota(iotac_i[:], pattern=[[0, 1]], base=0, channel_multiplier=1)
            iotac_f = sb.tile([P, 1], dt.float32)
            nc.vector.tensor_copy(out=iotac_f[:], in_=iotac_i[:])
            ones_b = sb.tile([P, P], dt.bfloat16)
            nc.vector.memset(ones_b[:], 1.0)
            ident_b = sb.tile([P, P], dt.bfloat16)
            make_identity(nc, ident_b[:])
            ident_f = sb.tile([P, P], dt.float32)
            make_identity(nc, ident_f[:])

            idx16_sb = sb.tile([P, TOTCH * 8], dt.int16)
            nc.sync.dma_start(out=idx16_sb[:], in_=i_idx16.ap())
            dla_sb = sb.tile([P, TOTCH], dt.float32)
            nc.sync.dma_start(out=dla_sb[:], in_=i_dla.ap())
            dlab_sb = sb.tile([P, TOTCH], dt.bfloat16)
            nc.vector.tensor_copy(out=dlab_sb[:], in_=dla_sb[:])

            RWR = regions[0]["rwr"]
            CHGC = RWR * CU
            # static [p, j, c] grid: value j, packed along c
            iotajc = sb.tile([P, P, CHGC], dt.bfloat16)
            nc.vector.tensor_copy(
                out=iotajc[:],
                in_=iota_b[:].rearrange("p (j o) -> p j o", o=1)
                    .to_broadcast([P, P, CHGC]))

            wg_sb = sb.tile([P, L * 2 * D], dt.bfloat16)
            for l in range(L):
                for h in range(2):
                    nc.sync.dma_start(out=wg_sb[:, (l * 2 + h) * D:(l * 2 + h + 1) * D],
                                      in_=i_wg.ap()[l, h])
            wast_sb = sb.tile([P, (L - 1) * 2], dt.bfloat16)
            for l in range(L - 1):
                nc.sync.dma_start(out=wast_sb[:, l * 2:l * 2 + 2], in_=i_wast.ap()[l])
            bg_sb = sb.tile([P, L], dt.float32)
            for l in range(L):
                nc.sync.dma_start(out=bg_sb[:, l:l + 1], in_=i_bg.ap()[l])

            xt_own = sb.tile([P, NSLAB], dt.bfloat16)
            tstages = [sb.tile([P, WPC], dt.float32, tag=f"tst{l}", name=f"tst{l}")
                       for l in range(L)]


            NO_COLL = int(os.environ.get("KGAT_NO_COLL", "0"))

            def allgather(src_t, dst_t):
                if NO_COLL == 2:
                    # light stub for TimelineSim: model only the local write;
                    # the real collective runs on TOPSP/SDMA, not our engines
                    nc.sync.dma_start(out=dst_t.ap()[0:NSLAB], in_=src_t.ap())
                elif NO_COLL:
                    for k in range(NCORES):
                        nc.sync.dma_start(
                            out=dst_t.ap()[k * NSLAB:(k + 1) * NSLAB], in_=src_t.ap())
                else:
                    nc.gpsimd.collective_compute(
                        "AllGather", mybir.AluOpType.bypass,
                        replica_groups=[list(range(NCORES))],
                        ins=[src_t.ap()], outs=[dst_t.ap()])

            for rep in range(REPS):
                nc.sync.dma_start(out=xt_own[:], in_=i_xt0.ap())
                nc.sync.dma_start(out=tstages[0][:], in_=i_t0.ap())
                nc.sync.dma_start(out=agin[0].ap(), in_=i_xslab.ap())
                allgather(agin[0], xfull[0])

                for l in range(L_RUN):
                    last = (l == L_RUN - 1)
                    xsrc = xfull[l]
                    tst = tstages[l]
                    if not last:
                        stage = sb.tile([P, WPC, ROWC], dt.bfloat16, tag="stage")
                        nc.vector.memset(stage[:, :, D + 1:], 0)
                    else:
                        stagef = sb.tile([P, WPC, D], dt.float32, tag="stage")

                    for reg in regions:
                        w0, w1, ch0, rwr = reg["w0"], reg["w1"], reg["ch0"], reg["rwr"]
                        CHG = rwr * CU
                        CHR = CHG * NQ
                        G = sbg.tile([P, CHR, ROWC], dt.bfloat16, tag="G")
                        for q in range(NQ):
                            cha = ch0 + q * CHG
                            nc.gpsimd.dma_gather(
                                G[:, q * CHG:(q + 1) * CHG, :],
                                xsrc.ap()[q * QROWS:(q + 1) * QROWS],
                                idx16_sb[:, cha * 8:(cha + CHG) * 8],
                                CHG * P, CHG * P, ROWC, single_packet=False)

                        # TBCJ[p, j, w] = t_w[j]: per-window rank-1 matmul then copy
                        TBCJ = sbg.tile([P, P, rwr], dt.bfloat16, tag="TBCJ")
                        for w in range(w0, w1):
                            diag = sbw.tile([P, P], dt.bfloat16, tag="diag")
                            nc.vector.tensor_scalar(
                                diag[:], iota_b[:], iotac_f[:, 0:1], tst[:, w:w + 1],
                                mybir.AluOpType.is_equal, mybir.AluOpType.mult)
                            tb = psT.tile([P, P], dt.float32, tag="tb")
                            nc.tensor.matmul(out=tb[:], lhsT=ones_b[:], rhs=diag[:],
                                             start=True, stop=True)
                            nc.scalar.activation(
                                out=TBCJ[:, :, w - w0:w - w0 + 1],
                                in_=tb[:].rearrange("p (j o) -> p j o", o=1),
                                func=mybir.ActivationFunctionType.Copy)

                        aggP = ps.tile([P, rwr * P], dt.float32, tag="agg")
                        Wts = []
                        for q in range(NQ):
                            base = q * CHG
                            gsl = G[:, base:base + CHG, :]
                            # s per chunk, packed [P, CHG]
                            scol = sbw.tile([P, CHG], dt.bfloat16, tag="scol")
                            nc.vector.tensor_copy(
                                out=scol[:],
                                in_=gsl[:, :, D:D + 1].rearrange("p c o -> p (c o)"))
                            OH = sbq.tile([P, P, CHG], dt.bfloat16, tag="OH")
                            nc.vector.tensor_tensor(
                                out=OH[:],
                                in0=iotajc[:, :, :CHG],
                                in1=dlab_sb[:, ch0 + base:ch0 + base + CHG]
                                    .rearrange("p (o c) -> p o c", o=1)
                                    .to_broadcast([P, P, CHG]),
                                op=mybir.AluOpType.is_equal)
                            PRE = sbq.tile([P, P, CHG], dt.bfloat16, tag="PRE")
                            nc.vector.tensor_tensor(
                                out=PRE[:].rearrange("p j (u w) -> p j u w", u=CU),
                                in0=TBCJ[:].rearrange("p j (o w) -> p j o w", o=1)
                                    .to_broadcast([P, P, CU, rwr]),
                                in1=scol[:].rearrange("p (o u w) -> p o u w", o=1, u=CU)
                                    .to_broadcast([P, P, CU, rwr]),
                                op=mybir.AluOpType.add)
                            SIG = sbq.tile([P, P, CHG], dt.bfloat16, tag="SIG")
                            nc.scalar.activation(
                                out=SIG[:], in_=PRE[:],
                                func=mybir.ActivationFunctionType.Sigmoid)
                            Wt = sbq.tile([P, P, CHG], dt.bfloat16, tag="W",
                                          bufs=NQ + 1)
                            nc.vector.tensor_tensor(
                                out=Wt[:], in0=OH[:], in1=SIG[:],
                                op=mybir.AluOpType.mult)
                            Wts.append((base, Wt))
                        for w in range(w0, w1):
                            for q in range(NQ):
                                base, Wt = Wts[q]
                                for u in range(CU):
                                    c = u * rwr + (w - w0)
                                    nc.tensor.matmul(
                                        out=aggP[:, (w - w0) * P:(w - w0 + 1) * P],
                                        lhsT=G[:, base + c, 0:D], rhs=Wt[:, :, c],
                                        start=(q == 0 and u == 0),
                                        stop=(q == NQ - 1 and u == CU - 1))

                        # ---- node updates for the region's windows ----
                        for w in range(w0, w1):
                            aggb = sbw.tile([P, P], dt.bfloat16, tag="aggb")
                            nc.scalar.activation(
                                out=aggb[:], in_=aggP[:, (w - w0) * P:(w - w0 + 1) * P],
                                func=mybir.ActivationFunctionType.Copy)
                            xts = xt_own[:, w * P:(w + 1) * P]
                            up = ps1.tile([P, P], dt.float32, tag="up")
                            nc.tensor.matmul(out=up[:],
                                             lhsT=wg_sb[:, (l * 2) * D:(l * 2 + 1) * D],
                                             rhs=xts, start=True, stop=False)
                            nc.tensor.matmul(out=up[:],
                                             lhsT=wg_sb[:, (l * 2 + 1) * D:(l * 2 + 2) * D],
                                             rhs=aggb[:], start=False, stop=True)
                            if not last:
                                nc.scalar.activation(out=xts, in_=up[:],
                                                     func=mybir.ActivationFunctionType.Relu,
                                                     bias=bg_sb[:, l:l + 1])
                                st = ps1.tile([P, 2], dt.float32, tag="st")
                                nc.tensor.matmul(out=st[:], lhsT=xts,
                                                 rhs=wast_sb[:, l * 2:l * 2 + 2],
                                                 start=True, stop=True)
                                tr = ps1.tile([P, P], dt.bfloat16, tag="tr")
                                nc.tensor.transpose(out=tr[:], in_=xts,
                                                    identity=ident_b[:])
                                nc.vector.tensor_copy(out=stage[:, w, 0:D], in_=tr[:])
                                nc.scalar.add(out=stage[:, w, D:D + 1], in_=st[:, 0:1],
                                              add=float(ba[l + 1, 0]))
                                nc.vector.tensor_copy(out=tstages[l + 1][:, w:w + 1],
                                                      in_=st[:, 1:2])
                            else:
                                xf = sbw.tile([P, P], dt.float32, tag="xf")
                                nc.scalar.activation(out=xf[:], in_=up[:],
                                                     func=mybir.ActivationFunctionType.Relu,
                                                     bias=bg_sb[:, l:l + 1])
                                trf = ps1.tile([P, P], dt.float32, tag="tr")
                                nc.tensor.transpose(out=trf[:], in_=xf[:],
                                                    identity=ident_f[:])
                                nc.vector.tensor_copy(out=stagef[:, w, :], in_=trf[:])

                    if not last:
                        nc.sync.dma_start(
                            out=agin[l + 1].ap().rearrange("(w p) c -> p w c", p=P),
                            in_=stage[:])
                        allgather(agin[l + 1], xfull[l + 1])
                    else:
                        nc.sync.dma_start(
                            out=o_out.ap().rearrange("(w p) c -> p w c", p=P),
                            in_=stagef[:])

    nc.compile()
    return nc


def kernel(edge_index, user_emb, item_emb, Wa, ba, Wg, bg):
    global LAST_EXEC_NS
    h = _host_prep(edge_index, user_emb, item_emb, Wa, ba, Wg, bg)
    nc = _build_nc(h["layout"], h["ba"])

    in_maps = []
    for k in range(NCORES):
        in_maps.append({
            "xslab": h["xslab"][k], "xt0": h["xt0"][k], "t0": h["t0c"][k],
            "idx16": h["idx16"][k], "dla": h["dla"][k],
            "wg": h["wg_b"], "wast": h["wast"], "bg": h["bg_c"],
        })

    res = run_bass_kernel_spmd(nc, in_maps, core_ids=list(range(NCORES)))
    LAST_EXEC_NS = res.exec_time_ns

    if int(os.environ.get("KGAT_BENCH", "0")):
        LAST_EXEC_NS = _bench(nc, in_maps)

    x = np.zeros((N, D), np.float32)
    for k in range(NCORES):
        x[k * NPC:(k + 1) * NPC] = np.asarray(res.results[k]["out"])[:NPC]
    return x[:U], x[U:]


def _bench(nc, in_maps, iters=None):
    """Time repeated on-device executions via the same PJRT shard_map path
    (device-resident inputs, no donation) and return min wall ns."""
    import time
    import jax
    from jax.sharding import Mesh, PartitionSpec
    from jax.experimental.shard_map import shard_map
    from concourse import bass2jax, mybir as mb

    if iters is None:
        iters = int(os.environ.get("KGAT_BENCH_ITERS", "10"))

    bass2jax.install_neuronx_cc_hook()
    partition_name = (nc.partition_id_tensor.name
                      if nc.partition_id_tensor else None)
    in_names, out_names, out_avals, zero_outs = [], [], [], []
    for alloc in nc.m.functions[0].allocations:
        if not isinstance(alloc, mb.MemoryLocationSet):
            continue
        name = alloc.memorylocations[0].name
        if alloc.kind == "ExternalInput":
            if name != partition_name:
                in_names.append(name)
        elif alloc.kind == "ExternalOutput":
            out_names.append(name)
            shape = tuple(alloc.tensor_shape)
            dtype = mb.dt.np(alloc.dtype)
            out_avals.append(jax.core.ShapedArray(shape, dtype))
            zero_outs.append(np.zeros(shape, dtype))
    n_params = len(in_names)
    all_names = in_names + out_names
    if partition_name is not None:
        all_names = all_names + [partition_name]

    def _body(*args):
        operands = list(args)
        if partition_name is not None:
            operands.append(bass2jax.partition_id_tensor())
        return tuple(bass2jax._bass_exec_p.bind(
            *operands, out_avals=tuple(out_avals), in_names=tuple(all_names),
            out_names=tuple(out_names), lowering_input_output_aliases=(),
            sim_require_finite=False, sim_require_nnan=False, nc=nc))

    devices = jax.devices()[:NCORES]
    mesh = Mesh(np.asarray(devices), ("core",))
    specs = (PartitionSpec("core"),) * (n_params + len(out_names))
    fn = jax.jit(shard_map(_body, mesh=mesh, in_specs=specs,
                           out_specs=(PartitionSpec("core"),) * len(out_names),
                           check_rep=False), keep_unused=True)
    concat_in = [np.concatenate([np.asarray(m[n]) for m in in_maps], axis=0)
                 for n in in_names]
    concat_zero = [np.zeros((NCORES * z.shape[0], *z.shape[1:]), z.dtype)
                   for z in zero_outs]
    sharding = jax.sharding.NamedSharding(mesh, PartitionSpec("core"))
    dev_in = [jax.device_put(a, sharding) for a in concat_in + concat_zero]
    jax.block_until_ready(fn(*dev_in))  # warm compile
    times = []
    for _ in range(iters):
        t0 = time.perf_counter()
        jax.block_until_ready(fn(*dev_in))
        times.append(time.perf_counter() - t0)
    times.sort()
    print(f"bench iters (ms): {[f'{t*1e3:.2f}' for t in times]}")
    return int(times[0] * 1e9)
